# revision 37
# baseline (speedup 1.0000x reference)
"""Single-head attention (B=4, S=4096, D=A=1024, fp32 I/O) on 8 TRN2 NeuronCores.

Sharding: core c handles batch b=c//2, sequence-half h=c%2 (2048 rows).

Algebraic folding removes half the projection work:
  scores = (x Wq^T + bq)(x Wk^T + bk)^T
         = (x M) x^T + (per-q const, cancels in softmax) + ck^T, M = Wq^T Wk,
         ck = x (Wk^T bq)  (host-computed, folded into the exp bias)
  out    = attn (x Wv^T + bv) Wo^T + bo
         = attn (x W'^T + Wo bv) + bo,                        W' = Wo Wv
so the device runs only TWO projections (q'' = x M and v' = x W'^T): the K
projection and the output projection are gone, and the pair-exchange for the
score matmuls ships raw x^T -- which is available at t=0, so the AllGather
chain starts immediately instead of waiting on K compute.

Core pairs exchange x^T and V' halves with chunked 2.1 MB AllGathers
(collective cost is super-linear in size; while one transfers, regular DMA
queues starve, so stores are buffered deep enough to ride out a window).
Attention runs per 512-query block against the full gathered sequence.

Device layout is transpose-free end to end: host passes x^T tiles and
pre-tiled folded weights; q''^T stays resident in SBUF; scores are computed
transposed ([k,q]); ctx accumulates as out^T and the DEVICE WRITES out^T --
the host transposes while unsharding.  Softmax normalization is deferred to
the epilogue: denominators accumulate on the vector engine (single
ones-matmul for the partition reduction), reciprocals are broadcast across
partitions with a rank-1 matmul, and the epilogue is one DVE multiply + bias
add per tile.  exp without max subtraction is safe: scores are O(5).
Matmul compute in bf16, accumulation fp32.  k-tiles are enumerated in gather
order everywhere, which keeps scores, exp, sums and ctx consistent without
any index remapping.
"""

import numpy as np
import ml_dtypes

import concourse.bass as bass
import concourse.tile as tile
from concourse import mybir
from concourse.bass_utils import run_bass_kernel_spmd

BF = mybir.dt.bfloat16
F32 = mybir.dt.float32
AF = mybir.ActivationFunctionType

B, S, DIM, A = 4, 4096, 1024, 1024
SQ = S // 2          # rows handled per core (query rows and local K/V rows)
NC = DIM // 128      # d chunks
NA = A // 128        # a tiles
NK = S // 128        # k tiles (global)
QB = 512             # q block width
NQB = SQ // QB
SB = SQ // 512       # 512-col blocks of the local sequence half
SCALE = 1.0 / np.sqrt(np.float32(A))

N_CORES = 8
PAIRS = [[0, 1], [2, 3], [4, 5], [6, 7]]

LAST_RESULT = None   # BassKernelResults of the most recent run (for test.py)


def _split_multiwaits(nc):
    """This walrus build rejects instructions carrying more than one sem wait
    (and Drains carrying any); hoist extra waits into single-wait NoOps
    preceding the instruction on the same engine."""
    for f in nc.m.functions:
        for bb in f.blocks:
            new_insts = []
            for inst in bb.instructions:
                si = inst.sync_info
                if si is not None and si.on_wait:
                    keep = 0 if isinstance(inst, mybir.InstDrain) else 1
                    if len(si.on_wait) > keep:
                        waits = list(si.on_wait)
                        hoist, rest = waits[: len(waits) - keep], waits[len(waits) - keep :]
                        for w in hoist:
                            nop = mybir.InstNoOp(
                                name=nc.get_next_instruction_name(),
                                sync_info=mybir.SyncInfo(on_wait=[w], on_update=[]),
                                bass_nofuse=True,
                                engine=inst.engine,
                            )
                            nc.register_instruction(nop)
                            new_insts.append(nop)
                        si.on_wait.clear()
                        si.on_wait.extend(rest)
                new_insts.append(inst)
            bb.instructions[:] = new_insts


def _build():
    nc = bass.Bass()

    # all inputs pre-tiled on host: [sb][128][c][cols] so device loads are
    # contiguous per partition
    xTq = nc.declare_dram_parameter("xTq", [SB, 128, NC, 512], BF, isOutput=False)
    WqT = nc.declare_dram_parameter("WqT", [128, NC, A], BF, isOutput=False)   # M = Wq^T Wk, tiled
    WvT = nc.declare_dram_parameter("WvT", [128, NC, A], BF, isOutput=False)   # W'^T = (Wo Wv)^T, tiled
    bvb = nc.declare_dram_parameter("bvb", [128, A], BF, isOutput=False)       # Wo bv, broadcast
    bob = nc.declare_dram_parameter("bob", [128, NC], F32, isOutput=False)     # bo, per-partition column
    cks = nc.declare_dram_parameter("cks", [128, NK], F32, isOutput=False)     # x(Wk^T bq)/32, gather order
    outT = nc.declare_dram_parameter("outT", [DIM, SQ], F32, isOutput=True)

    with tile.TileContext(nc) as tc:
        with (
            tc.tile_pool(name="dram", bufs=1, space="DRAM") as dram,
            tc.tile_pool(name="singles", bufs=1) as singles,
        ):
            # collective buffers, chunked at 2.1 MB.  xq ships raw x^T in the
            # same tiled layout the score matmuls stream it back in.
            xq_in = [
                dram.tile([2, 128, NC, 512], BF, name=f"xq_in{c}", tag=f"xqi{c}")
                for c in range(2)
            ]
            xq_out = [
                dram.tile([2, 2, 128, NC, 512], BF, name=f"xq_out{c}", tag=f"xqo{c}")
                for c in range(2)
            ]
            v_in = [
                dram.tile([1024, A], BF, name=f"v_in{c}", tag=f"vi{c}")
                for c in range(2)
            ]
            v_out = [
                dram.tile([2, 1024, A], BF, name=f"v_out{c}", tag=f"vo{c}")
                for c in range(2)
            ]

            warm_in = dram.tile([1, 128], BF, name="warm_in")
            warm_out = dram.tile([2, 1, 128], BF, name="warm_out")

            v_sb = singles.tile([128, NK, A], BF)        # V' resident, 8.4 MB
            qt_sb = singles.tile([128, NA, SQ], BF)      # q''^T resident, 4.2 MB
            cks_sb = singles.tile([128, NK], F32)
            bob_sb = singles.tile([128, NC], F32)
            ones_k = singles.tile([128, 1], F32)         # sums matmul lhsT
            ones_r = singles.tile([1, 128], F32)         # recip broadcast lhsT

            # phase-2 streaming pool allocated BEFORE the phase-1 pools so
            # its SBUF addresses are disjoint from phase-1 tiles -> its
            # prefetch DMAs carry no WAR dependency on phase-1 compute
            ksp = tc.tile_pool(name="p2k", bufs=4)
            p2k = ksp.__enter__()

            # ---------------- Phase 1: projections + x^T/V' exchange -------
            with (
                tc.tile_pool(name="p1w", bufs=1) as p1w,
                tc.tile_pool(name="p1x", bufs=1) as p1x,
                tc.tile_pool(name="p1b", bufs=1) as p1b,
                tc.tile_pool(name="p1vo", bufs=6) as p1vo,
                tc.tile_pool(name="p1pq", bufs=2, space="PSUM") as p1pq,
                tc.tile_pool(name="p1pv", bufs=2, space="PSUM") as p1pv,
            ):
                wq = p1w.tile([128, NC, A], BF, tag="wq")
                wv = p1w.tile([128, NC, A], BF, tag="wv")
                bvb_sb = p1b.tile([128, A], BF)
                # all of x^T stays resident through phase 1 so no PE input
                # depends on DMA while the collectives are saturating HBM
                xs_all = p1x.tile([128, SB, NC, 512], BF)

                # wake the collectives firmware immediately (the first
                # collective otherwise pays ~25us of startup latency at the
                # head of the exchange chain); staged through an internal
                # DRAM tile since collectives can't read I/O tensors
                nc.gpsimd.dma_start(out=warm_in[:], in_=xTq[0, 0:1, 0, 0:128])
                nc.gpsimd.collective_compute(
                    "AllGather",
                    mybir.AluOpType.bypass,
                    replica_groups=PAIRS,
                    ins=[warm_in[:].opt()],
                    outs=[warm_out[:].opt()],
                )
                # minimal DMA before the first matmul (split loads so dc=0
                # matmuls start early)
                nc.sync.dma_start(out=wv[:, 0:2, :], in_=WvT[:, 0:2, :])
                nc.scalar.dma_start(out=xs_all[:, 0, 0:2, :], in_=xTq[0, :, 0:2, :])
                nc.scalar.dma_start(out=wv[:, 2:8, :], in_=WvT[:, 2:8, :])
                nc.scalar.dma_start(out=bvb_sb[:], in_=bvb[:])
                nc.sync.dma_start(out=xs_all[:, 0, 2:8, :], in_=xTq[0, :, 2:8, :])
                for sb in range(1, SB):
                    nc.sync.dma_start(out=xs_all[:, sb, :, :], in_=xTq[sb])
                nc.sync.dma_start(out=wq[:], in_=WqT[:])
                nc.scalar.dma_start(out=cks_sb[:], in_=cks[:])
                nc.scalar.dma_start(out=bob_sb[:], in_=bob[:])

                # x^T bounce copies (collectives can't read I/O tensors) and
                # their AllGathers.  No compute dependency, but queued on the
                # HWDGE rings BEHIND the critical SBUF loads so they don't
                # steal SDMA bandwidth from the projection startup; the
                # AllGathers are gated by the warmup collective anyway.
                for c in range(2):
                    nc.sync.dma_start(out=xq_in[c][0], in_=xTq[2 * c])
                    nc.scalar.dma_start(out=xq_in[c][1], in_=xTq[2 * c + 1])
                    nc.gpsimd.collective_compute(
                        "AllGather",
                        mybir.AluOpType.bypass,
                        replica_groups=PAIRS,
                        ins=[xq_in[c][:].opt()],
                        outs=[xq_out[c][:].opt()],
                    )

                def v_chunk(c):
                    for sbl in range(2):
                        sb = c * 2 + sbl
                        for st in range(4):
                            pv = p1pv.tile([128, 1024], F32)
                            for half in range(2):
                                for dc in range(NC):
                                    nc.tensor.matmul(
                                        pv[:, half * 512 : (half + 1) * 512],
                                        lhsT=xs_all[:, sb, dc, st * 128 : (st + 1) * 128],
                                        rhs=wv[:, dc, half * 512 : (half + 1) * 512],
                                        start=(dc == 0),
                                        stop=(dc == NC - 1),
                                    )
                            vo = p1vo.tile([128, 1024], BF, tag="vo")
                            nc.vector.tensor_add(vo[:], pv[:], bvb_sb[:])
                            nc.scalar.dma_start(
                                out=v_in[c][
                                    (sbl * 4 + st) * 128 : (sbl * 4 + st + 1) * 128, :
                                ],
                                in_=vo[:],
                            )
                    nc.gpsimd.collective_compute(
                        "AllGather",
                        mybir.AluOpType.bypass,
                        replica_groups=PAIRS,
                        ins=[v_in[c][:].opt()],
                        outs=[v_out[c][:].opt()],
                    )

                def v_sb_load(c):
                    # gathered V' -> resident SBUF, k enumerated in gather order
                    for hh in range(2):
                        nc.gpsimd.dma_start(
                            out=v_sb[:, c * 16 + hh * 8 : c * 16 + hh * 8 + 8, :],
                            in_=v_out[c][hh].rearrange("(j p) a -> p j a", p=128),
                        )

                v_chunk(0)
                v_sb_load(0)
                v_chunk(1)
                v_sb_load(1)

                # --- q'' projection (overlaps the V' exchanges); writes
                # directly into resident SBUF, no DRAM staging, no bias ---
                for qb in range(NQB):
                    for am in range(NA):
                        pq = p1pq.tile([128, 512], F32)
                        for dc in range(NC):
                            nc.tensor.matmul(
                                pq[:],
                                lhsT=wq[:, dc, am * 128 : (am + 1) * 128],
                                rhs=xs_all[:, qb, dc, :],
                                start=(dc == 0),
                                stop=(dc == NC - 1),
                            )
                        nc.scalar.activation(
                            qt_sb[:, am, qb * 512 : (qb + 1) * 512],
                            pq[:],
                            AF.Identity,
                        )

                nc.vector.memset(ones_k[:], 1.0)
                nc.vector.memset(ones_r[:], 1.0)

            # ---------------- Phase 2: attention ----------------
            with (
                tc.tile_pool(name="p2e", bufs=1) as p2e,
                tc.tile_pool(name="p2c", bufs=1) as p2c,
                tc.tile_pool(name="p2a", bufs=2) as p2a,
                tc.tile_pool(name="p2s", bufs=1) as p2s,
                tc.tile_pool(name="p2r", bufs=2) as p2r,
                tc.tile_pool(name="p2o", bufs=2) as p2o,
                tc.tile_pool(name="pps", bufs=3, space="PSUM") as pps,
                tc.tile_pool(name="ppsum", bufs=1, space="PSUM") as ppsum,
                tc.tile_pool(name="pprb", bufs=1, space="PSUM") as pprb,
                tc.tile_pool(name="ppc", bufs=3, space="PSUM") as ppc,
            ):
                # exp tiles live in a 48-slot ring (1.5 q-blocks): block qb's
                # k-tile kt sits at slot (32*qb + kt) % 48.  The pipeline
                # emits the next block's scores in two halves (ctx-A half
                # after ctxA, ctx-B half after the epilogue), so every ring
                # overwrite lands on slices whose reader already retired.
                et_ring = p2e.tile([128, 48, QB], BF, name="et_ring")

                def slot(qb, kt):
                    return (32 * qb + kt) % 48

                def do_scores_half(qb, c, acc):
                    # scores^T + exp for gather-chunk c (16 k-tiles); k-tile
                    # groups of 4 share one x^T load.  Denominators
                    # accumulate on the vector engine alongside.
                    for hh in range(2):
                        for half in range(2):
                            ks = p2k.tile([128, NC, 512], BF, name=f"ks{qb}_{c}{hh}{half}", tag="ks")
                            nc.sync.dma_start(
                                out=ks[:],
                                in_=xq_out[c][hh, half],
                            )
                            ebase = c * 16 + hh * 8 + half * 4
                            for kt4 in range(4):
                                ps = pps.tile([128, QB], F32, name=f"ps{qb}_{ebase+kt4}", tag="ps")
                                for ac in range(NC):
                                    nc.tensor.matmul(
                                        ps[:],
                                        lhsT=ks[:, ac, kt4 * 128 : (kt4 + 1) * 128],
                                        rhs=qt_sb[:, ac, qb * QB : (qb + 1) * QB],
                                        start=(ac == 0),
                                        stop=(ac == NC - 1),
                                    )
                                kt = ebase + kt4
                                nc.scalar.activation(
                                    et_ring[:, slot(qb, kt), :],
                                    ps[:],
                                    AF.Exp,
                                    bias=cks_sb[:, kt : kt + 1],
                                    scale=float(SCALE),
                                )
                                if kt == 0:
                                    nc.vector.tensor_copy(
                                        acc[:], et_ring[:, slot(qb, 0), :]
                                    )
                                else:
                                    nc.vector.tensor_add(
                                        acc[:], acc[:], et_ring[:, slot(qb, kt), :]
                                    )

                def do_rowsum(qb, acc):
                    # single partition-reduction matmul over the accumulated
                    # exp sums, then reciprocal on the [1, QB] row
                    p_row = ppsum.tile([1, QB], F32, name=f"p_row{qb}", tag="p_row")
                    nc.tensor.matmul(
                        p_row[:],
                        lhsT=ones_k[:, 0:1],
                        rhs=acc[:],
                        start=True,
                        stop=True,
                    )
                    srec = p2s.tile([1, QB], F32, name=f"srec{qb}", tag="srec")
                    nc.vector.reciprocal(srec[:], p_row[:])
                    return srec

                def do_rb(qb, srec):
                    # broadcast the reciprocal row across all 128 partitions
                    # with a rank-1 matmul
                    prb = pprb.tile([128, QB], F32, name=f"prb{qb}", tag="prb")
                    nc.tensor.matmul(
                        prb[:],
                        lhsT=ones_r[0:1, :],
                        rhs=srec[:],
                        start=True,
                        stop=True,
                    )
                    rb = p2r.tile([128, QB], F32, name=f"rb{qb}", tag="rb")
                    nc.vector.tensor_copy(rb[:], prb[:])
                    return rb

                def do_ctxA(qb):
                    # first gather half of ctx^T (= out^T, pre-normalization)
                    ct = p2c.tile([128, NA, QB], BF, name=f"ct{qb}", tag="ct")
                    for at in range(NA):
                        pc = ppc.tile([128, QB], F32, name=f"pcA{qb}_{at}", tag="pc")
                        for kt in range(NK // 2):
                            nc.tensor.matmul(
                                pc[:],
                                lhsT=v_sb[:, kt, at * 128 : (at + 1) * 128],
                                rhs=et_ring[:, slot(qb, kt), :],
                                start=(kt == 0),
                                stop=(kt == NK // 2 - 1),
                            )
                        nc.vector.tensor_copy(ct[:, at, :], pc[:])
                    return ct

                def do_ctxB(qb, ct):
                    for at in range(NA):
                        pc = ppc.tile([128, QB], F32, name=f"pcB{qb}_{at}", tag="pc")
                        for kt in range(NK // 2, NK):
                            nc.tensor.matmul(
                                pc[:],
                                lhsT=v_sb[:, kt, at * 128 : (at + 1) * 128],
                                rhs=et_ring[:, slot(qb, kt), :],
                                start=(kt == NK // 2),
                                stop=(kt == NK - 1),
                            )
                        nc.vector.tensor_add(ct[:, at, :], pc[:], ct[:, at, :])
                    return ct

                def do_fin(qb, ct, rb):
                    # deferred softmax normalization (broadcast reciprocal
                    # row) + output bias; store out^T tiles
                    for at in range(NA):
                        ob = p2o.tile([128, QB], F32, name=f"ob{qb}_{at}", tag="ob")
                        nc.vector.tensor_mul(ob[:], ct[:, at, :], rb[:])
                        nc.vector.tensor_scalar(
                            ob[:],
                            ob[:],
                            bob_sb[:, at : at + 1],
                            None,
                            op0=mybir.AluOpType.add,
                        )
                        nc.sync.dma_start(
                            out=outT[
                                at * 128 : (at + 1) * 128,
                                qb * QB : (qb + 1) * QB,
                            ],
                            in_=ob[:],
                        )

                # software pipeline: the next block's scores are emitted in
                # two halves -- ctx-A half between ctxA and ctxB, ctx-B half
                # after the epilogue -- so the PE always has independent work
                # while the current block's ctx chain settles, and the et
                # ring overwrites only retired slices.
                acc = p2a.tile([128, QB], F32, name="acc0", tag="acc")
                do_scores_half(0, 0, acc)
                do_scores_half(0, 1, acc)
                acc_next = None
                for qb in range(NQB):
                    srec = do_rowsum(qb, acc)
                    ct = do_ctxA(qb)
                    rb = do_rb(qb, srec)
                    if qb + 1 < NQB:
                        acc_next = p2a.tile([128, QB], F32, name=f"acc{qb+1}", tag="acc")
                        do_scores_half(qb + 1, 0, acc_next)
                    ct = do_ctxB(qb, ct)
                    do_fin(qb, ct, rb)
                    if qb + 1 < NQB:
                        do_scores_half(qb + 1, 1, acc_next)
                    acc = acc_next
            ksp.__exit__(None, None, None)

    _split_multiwaits(nc)
    return nc


_NC_CACHE = None


def _get_nc():
    global _NC_CACHE
    if _NC_CACHE is None:
        _NC_CACHE = _build()
    return _NC_CACHE


def kernel(x, Wq, bq, Wk, bk, Wv, bv, Wo, bo):
    global LAST_RESULT
    bf16 = ml_dtypes.bfloat16
    x = np.asarray(x, np.float32)
    Wq = np.asarray(Wq, np.float32)
    Wk = np.asarray(Wk, np.float32)
    Wv = np.asarray(Wv, np.float32)
    Wo = np.asarray(Wo, np.float32)
    bq = np.asarray(bq, np.float32)
    bv = np.asarray(bv, np.float32)
    bo = np.asarray(bo, np.float32)

    def tile128(m):
        # [R, C] with R = 128*nc -> [128, nc, C], row r=c*128+p -> (p, c)
        R, C = m.shape
        return np.ascontiguousarray(
            m.reshape(R // 128, 128, C).transpose(1, 0, 2)
        )

    M = Wq.T @ Wk                       # q'' = x @ M
    Wp = Wo @ Wv                        # v' = x @ Wp^T + Wo bv
    ckv = Wk.T @ bq                     # per-k score offset direction
    WqTt = tile128(M.astype(bf16))
    WvTt = tile128(np.ascontiguousarray(Wp.T).astype(bf16))
    bvb = np.ascontiguousarray(
        np.broadcast_to(Wo @ bv, (128, A))
    ).astype(bf16)
    bob = np.ascontiguousarray(bo.reshape(NC, 128).T)

    in_maps = []
    for c in range(N_CORES):
        b, h = c // 2, c % 2
        xT = x[b, h * SQ : (h + 1) * SQ, :].T.astype(bf16)
        # [DIM, SQ] -> [sb][128][c-chunk][512]
        xT4 = np.ascontiguousarray(
            np.asarray(xT, bf16).reshape(NC, 128, SB, 512).transpose(2, 1, 0, 3)
        )
        # per-k exp bias in this core's gather order: chunk-major over the
        # pair's halves, matching xq_out/v_sb tile order
        pair = PAIRS[c // 2]
        ckf = np.empty((128, NK), np.float32)
        for cc in range(2):
            for hh in range(2):
                xh = x[b, pair[hh] % 2 * SQ : (pair[hh] % 2) * SQ + SQ, :]
                cv = (xh[cc * 1024 : (cc + 1) * 1024] @ ckv) / 32.0
                for j in range(8):
                    ckf[:, cc * 16 + hh * 8 + j] = cv[j * 128 : (j + 1) * 128]
        in_maps.append(
            {
                "xTq": xT4,
                "WqT": WqTt,
                "WvT": WvTt,
                "bvb": bvb,
                "bob": bob,
                "cks": ckf,
            }
        )

    nc = _get_nc()
    import os

    # NTFF tracing needs the antenv.axon_hooks shim; without it the trace
    # branch of run_bass_kernel_spmd raises ImportError.  Only trace when
    # both requested and available, and pin BASS_NEVER_TRACE otherwise so
    # the env var alone can't re-enable the broken path.
    trace = bool(os.environ.get("BASS_TRACE"))
    if trace:
        try:
            import antenv.axon_hooks  # noqa: F401
        except ImportError:
            trace = False
    if not trace:
        os.environ["BASS_NEVER_TRACE"] = "1"

    res = run_bass_kernel_spmd(
        nc,
        in_maps,
        core_ids=list(range(N_CORES)),
        trace=trace,
    )
    LAST_RESULT = res

    out_full = np.empty((B, S, DIM), np.float32)
    for c in range(N_CORES):
        b, h = c // 2, c % 2
        out_full[b, h * SQ : (h + 1) * SQ, :] = res.results[c]["outT"].T
    return out_full
